# revision 1
# baseline (speedup 1.0000x reference)
"""Multi-head attention block kernel for Trainium2 (8 NeuronCores).

Problem: x:(2,4,1024,512) fp32, W_qkv:(512,3072), b_qkv:(3072,),
W_out:(1024,512), b_out:(512,).  out = Attention(x) per (bt,b) item.

Sharding: pure data parallel — bt*b_sz = 8 batch items, one per core.
Each core runs the full attention block on its (1024, 512) slice:
  qkv = x @ W_qkv + b_qkv           (heads=8, hd=128; scale=1/sqrt(64))
  P   = softmax(q*scale @ k^T)
  o   = (P @ v) reshaped, then o @ W_out + b_out

On-chip plan (all matmuls bf16 with fp32 PSUM accumulation):
  - x cast to bf16 (DMA cast), x^T built with DMA-xbar transposes
  - V GEMM first (Form A: n on partitions), bias via K=1 ones-row matmul,
    heads side by side with a ones column appended per head
  - then a per-head software pipeline so ScalarE exp overlaps TensorE:
      q^T,k^T projection (Form B, hd on partitions) -> S^T = k^T^T q^T
      -> P^T = exp(0.125 S^T) on ScalarE -> attn out per query-chunk with
      rhs [v | ones] (col 128 of PSUM = softmax denominator) -> reciprocal
      normalize (queries on partitions) -> PE-transpose to (h*hd, n)
  - final = out_cat^T-slices^T @ W_out + b_out (ones-row matmul bias),
    output DMA per row-chunk overlapped with the GEMM
"""

import numpy as np

P = 128
N_CTX = 1024
DIM = 512
H = 8
HD = 128
QKV = 3072
SCALE = 0.125  # (512 // 8) ** -0.5, faithful to the reference

_cached_nc = None


def _build_nc(loop_n=1):
    from contextlib import ExitStack

    import concourse.mybir as mybir
    import concourse.tile as tile
    from concourse import bacc
    from concourse.masks import make_identity

    F32 = mybir.dt.float32
    BF16 = mybir.dt.bfloat16
    AF = mybir.ActivationFunctionType

    nc = bacc.Bacc()

    x_ext = nc.declare_dram_parameter("x", [N_CTX, DIM], F32, isOutput=False)
    wqkv_ext = nc.declare_dram_parameter("W_qkv", [DIM, QKV], F32, isOutput=False)
    bqkv_ext = nc.declare_dram_parameter("b_qkv", [QKV], F32, isOutput=False)
    wout_ext = nc.declare_dram_parameter("W_out", [N_CTX, DIM], F32, isOutput=False)
    bout_ext = nc.declare_dram_parameter("b_out", [DIM], F32, isOutput=False)
    out_ext = nc.declare_dram_parameter("out", [N_CTX, DIM], F32, isOutput=True)

    NT = N_CTX // P  # 8 row tiles
    KD = DIM // P  # 4 contraction chunks for dim=512
    VW = HD + 1  # 129: v columns per head incl. ones column

    with ExitStack() as ctx:
        tc = ctx.enter_context(tile.TileContext(nc))
        consts = ctx.enter_context(tc.tile_pool(name="consts", bufs=1))
        persist = ctx.enter_context(tc.tile_pool(name="persist", bufs=1))
        work = ctx.enter_context(tc.tile_pool(name="work", bufs=2))
        small = ctx.enter_context(tc.tile_pool(name="small", bufs=3))
        dram = ctx.enter_context(tc.tile_pool(name="dram", bufs=1, space="DRAM"))
        ps_big = ctx.enter_context(tc.tile_pool(name="ps_big", bufs=2, space="PSUM"))
        ps_bank = ctx.enter_context(tc.tile_pool(name="ps_bank", bufs=4, space="PSUM"))

        # ---- constants / weights (outside any bench loop) -------------------
        ident = consts.tile([P, P], BF16, tag="ident")
        make_identity(nc, ident)
        ones_row = consts.tile([1, P], BF16, tag="ones_row")
        nc.vector.memset(ones_row, 1.0)

        # x via HWDGE fp32 (sync queue), cast + PE-transposed on chip —
        # keeps the gpsimd (SWDGE cast) queue free for the weight loads
        x_sb = persist.tile([P, NT, DIM], F32, tag="x_sb")
        for t in range(NT):
            nc.sync.dma_start(
                x_sb[:, t, :], x_ext.rearrange("(t p) d -> p t d", p=P)[:, t, :]
            )

        # W_qkv as (p, ko, 3072) bf16 — contraction dim on partitions.
        # v columns first (the V GEMM runs first), then q|k per k-chunk.
        wq_sb = consts.tile([P, KD, QKV], BF16, tag="wq")
        wq_r = wqkv_ext.rearrange("(ko p) n -> p ko n", p=P)
        for k in range(KD):
            nc.gpsimd.dma_start(wq_sb[:, k, 2 * H * P :], wq_r[:, k, 2 * H * P :])
        for part in range(2):  # 0: q cols, 1: k cols
            for k in range(KD):
                sl = slice(part * H * P, (part + 1) * H * P)
                nc.gpsimd.dma_start(wq_sb[:, k, sl], wq_r[:, k, sl])
        # W_out as (p, kh, 512) bf16 — contraction dim (h*hd) on partitions
        wout_sb = consts.tile([P, H, DIM], BF16, tag="wout")
        nc.gpsimd.dma_start(wout_sb, wout_ext.rearrange("(kh p) c -> p kh c", p=P))
        # q/k bias in partition-major layout: bqk[p, m] = b_qkv[m*128 + p]
        bqk_sb = consts.tile([P, 2 * H], F32, tag="bqk")
        nc.sync.dma_start(
            bqk_sb, bqkv_ext[0 : 2 * H * P].rearrange("(t p) -> p t", p=P)
        )
        # v bias and out bias as single-partition rows (bf16, for K=1 matmuls)
        bv_row = consts.tile([1, H * HD], BF16, tag="bv")
        nc.gpsimd.dma_start(bv_row, bqkv_ext[2 * H * P : QKV][None, :])
        bout_row = consts.tile([1, DIM], BF16, tag="bout")
        nc.gpsimd.dma_start(bout_row, bout_ext[None, :])

        def body(_iv=None):
            # ---- x^T: cast to bf16 on DVE, transpose 128x128 blocks on PE --
            x_bf = persist.tile([P, NT, DIM], BF16, tag="x_bf")
            for t in range(NT):
                nc.vector.tensor_copy(x_bf[:, t, :], x_sb[:, t, :])
            xT = persist.tile([P, KD, N_CTX], BF16, tag="xT")
            for t in range(NT):
                for c in range(KD):
                    tp = ps_bank.tile([P, P], BF16, tag="bank")
                    nc.tensor.transpose(tp, x_bf[:, t, c * P : (c + 1) * P], ident)
                    nc.vector.tensor_copy(xT[:, c, t * P : (t + 1) * P], tp)

            # ---- v first (Form A): n on partitions, heads side by side with
            # a ones column: v_sb[:, t, h*129+128] = 1.0 -> softmax sums ride
            # along in the attention matmul for free.
            v_sb = persist.tile([P, NT, H * VW], BF16, tag="v_sb")
            nc.vector.memset(
                v_sb.rearrange("p t (h w) -> p t h w", w=VW)[:, :, :, HD : HD + 1],
                1.0,
            )
            for t in range(NT):
                for half in range(2):
                    ps = ps_bank.tile([P, DIM], F32, tag="bank")
                    for k in range(KD):
                        nc.tensor.matmul(
                            ps,
                            xT[:, k, t * P : (t + 1) * P],
                            wq_sb[
                                :,
                                k,
                                2 * H * P + half * DIM : 2 * H * P + (half + 1) * DIM,
                            ],
                            start=(k == 0),
                            stop=False,
                        )
                    nc.tensor.matmul(
                        ps,
                        ones_row,
                        bv_row[:, half * DIM : (half + 1) * DIM],
                        start=False,
                        stop=True,
                    )
                    dst = v_sb[:, t, :].rearrange("p (h w) -> p h w", w=VW)[
                        :, half * 4 : (half + 1) * 4, 0:HD
                    ]
                    src = ps.rearrange("p (h w) -> p h w", w=HD)
                    nc.vector.tensor_copy(dst, src)

            # ---- per-head software pipeline --------------------------------
            # Engines execute their scheduled streams in-order, so the
            # EMISSION order is the schedule.  Interleave head h's scores
            # (whose PSUM slots recycle at ScalarE's exp pace) with head
            # h-1's attention matmuls so the PE never waits inline on exp;
            # the final GEMM interleaves with the last head's attention.
            # q^T of head h = Form B M-tile over qkv cols h*128..(h+1)*128,
            # k^T of head h = cols 1024+h*128... (hd on partitions).
            outT = persist.tile([P, H, N_CTX], BF16, tag="outT")
            out_sb = persist.tile([P, NT, DIM], F32, tag="out_sb")
            out_r = out_ext.rearrange("(t p) c -> p t c", p=P)

            def emit_qk(h):
                pair = []
                for part in range(2):  # 0: q, 1: k
                    m = part * H + h
                    qk = work.tile([P, N_CTX], BF16, tag=f"qkT{part}")
                    for half in range(2):
                        sl = slice(half * DIM, (half + 1) * DIM)
                        ps = ps_bank.tile([P, DIM], F32, tag="bank")
                        for k in range(KD):
                            nc.tensor.matmul(
                                ps,
                                wq_sb[:, k, m * P : (m + 1) * P],
                                xT[:, k, sl],
                                start=(k == 0),
                                stop=(k == KD - 1),
                            )
                        nc.vector.tensor_scalar_add(
                            qk[:, sl], ps, bqk_sb[:, m : m + 1]
                        )
                    pair.append(qk)
                return pair

            def emit_scores_j(qkT_pair, pT, j):
                qT_h, kT_h = qkT_pair
                ps = ps_big.tile([P, N_CTX], F32, tag="big")
                for half in range(2):
                    sl = slice(half * DIM, (half + 1) * DIM)
                    nc.tensor.matmul(
                        ps[:, sl],
                        kT_h[:, j * P : (j + 1) * P],
                        qT_h[:, sl],
                        start=True,
                        stop=True,
                    )
                nc.scalar.activation(pT[:, j, :], ps, AF.Exp, scale=SCALE)

            def emit_attn_ic(h, pT, ic):
                aps = ps_bank.tile([P, VW], F32, tag="bank")
                for j in range(NT):
                    nc.tensor.matmul(
                        aps[:, :VW],
                        pT[:, j, ic * P : (ic + 1) * P],
                        v_sb[:, j, h * VW : (h + 1) * VW],
                        start=(j == 0),
                        stop=(j == NT - 1),
                    )
                rc = small.tile([P, 1], F32, tag="rc")
                nc.vector.reciprocal(rc, aps[:, HD : HD + 1])
                at = small.tile([P, P], BF16, tag="at")
                nc.vector.tensor_scalar_mul(at, aps[:, 0:HD], rc)
                return at

            def emit_transp(h, ic, at):
                tp = ps_bank.tile([P, P], BF16, tag="bank")
                nc.tensor.transpose(tp, at, ident)
                nc.vector.tensor_copy(outT[:, h, ic * P : (ic + 1) * P], tp)

            def emit_final_ic(ic):
                fps = ps_bank.tile([P, DIM], F32, tag="bank")
                for kh in range(H):
                    nc.tensor.matmul(
                        fps,
                        outT[:, kh, ic * P : (ic + 1) * P],
                        wout_sb[:, kh, :],
                        start=(kh == 0),
                        stop=False,
                    )
                nc.tensor.matmul(fps, ones_row, bout_row, start=False, stop=True)
                nc.vector.tensor_copy(out_sb[:, ic, :], fps)
                nc.sync.dma_start(out_r[:, ic, :], out_sb[:, ic, :])

            qk_prev = emit_qk(0)
            pT_prev = work.tile([P, NT, N_CTX], BF16, tag="pT")
            for j in range(NT):
                emit_scores_j(qk_prev, pT_prev, j)
            for h in range(1, H + 1):
                if h < H:
                    qk_cur = emit_qk(h)
                    pT_cur = work.tile([P, NT, N_CTX], BF16, tag="pT")
                at_prev = None
                for j in range(NT):
                    if h < H:
                        emit_scores_j(qk_cur, pT_cur, j)
                    # attention of the previous head fills the exp latency
                    at = emit_attn_ic(h - 1, pT_prev, j)
                    if at_prev is not None:
                        emit_transp(h - 1, j - 1, at_prev)
                    at_prev = at
                    if h == H and j >= 1:
                        emit_final_ic(j - 1)
                emit_transp(h - 1, NT - 1, at_prev)
                if h == H:
                    emit_final_ic(NT - 1)
                if h < H:
                    qk_prev, pT_prev = qk_cur, pT_cur

        if loop_n == 1:
            body()
        else:
            with tc.For_i(0, loop_n, 1) as iv:
                body(iv)

    nc.finalize()
    return nc


def _get_nc():
    global _cached_nc
    if _cached_nc is None:
        _cached_nc = _build_nc()
    return _cached_nc


def kernel(**inputs):
    from concourse.bass_utils import run_bass_kernel_spmd

    x = np.ascontiguousarray(np.asarray(inputs["x"], dtype=np.float32))
    W_qkv = np.ascontiguousarray(np.asarray(inputs["W_qkv"], dtype=np.float32))
    b_qkv = np.ascontiguousarray(np.asarray(inputs["b_qkv"], dtype=np.float32))
    W_out = np.ascontiguousarray(np.asarray(inputs["W_out"], dtype=np.float32))
    b_out = np.ascontiguousarray(np.asarray(inputs["b_out"], dtype=np.float32))

    bt, b_sz, n, dim = x.shape
    xs = x.reshape(bt * b_sz, n, dim)
    nc = _get_nc()
    in_maps = [
        {
            "x": np.ascontiguousarray(xs[c]),
            "W_qkv": W_qkv,
            "b_qkv": b_qkv,
            "W_out": W_out,
            "b_out": b_out,
        }
        for c in range(8)
    ]
    res = run_bass_kernel_spmd(nc, in_maps, core_ids=list(range(8)))
    outs = np.stack([np.asarray(res.results[c]["out"]) for c in range(8)])
    return outs.reshape(bt, b_sz, n, dim).astype(np.float32)



# revision 8
# speedup vs baseline: 2.9870x; 2.9870x over previous
"""Multi-head attention block kernel for Trainium2 (8 NeuronCores).

Problem: x:(2,4,1024,512) fp32, W_qkv:(512,3072), b_qkv:(3072,),
W_out:(1024,512), b_out:(512,).  out = Attention(x) per (bt,b) item.

Sharding: pure data parallel — bt*b_sz = 8 batch items, one per core.
Each core runs the full attention block on its (1024, 512) slice:
  qkv = x @ W_qkv + b_qkv           (heads=8, hd=128; scale=1/sqrt(64))
  P   = softmax(q*scale @ k^T)
  o   = (P @ v) reshaped, then o @ W_out + b_out

On-chip plan (all matmuls bf16 with fp32 PSUM accumulation).  HW-measured
rates on this part: N=512 MM ~199ns, N=129 MM ~82ns, exp[128,1024] ~1.2us,
and a PE-transpose->DVE-copy round trip costs >1us when the input is hot
(cross-engine ping-pong), but streams cleanly when the input is a phase old.
Schedule (emission order IS the schedule; engines run in-order):
  - fill: x cast (DVE) -> x^T PE-transposed 4 blocks per PSUM bank, one wide
    DVE copy per bank; q^T,k^T of head 0; scores+exp of head 0; THEN the
    V GEMM (its 16us of PE work hides head-0's 8 exps).  W_qkv DMA loads
    q|k columns first so head 0 can start.  V layout: heads side by side
    with a ones column appended (v | 1) -> the attention matmul's column
    128 accumulates the softmax denominator for free.
  - steady phase h: per query-chunk j: scores S^T_j(h) (2 N=512 MMs) ->
    exp on ScalarE; AV chain of head h-1 (8 N=129 MMs, rhs [v|1]) -> DVE
    reciprocal + normalize into at_store[h-1]; PE-transpose of head h-2's
    at_store (input a full phase old -> no ping-pong) -> DVE copy to outT;
    head h+1's q^T,k^T projection MMs spread across the phase (4-MM groups
    after each even j) so its DVE bias-adds land a phase early.
  - drain: transposes of head 7, then final = outT^T @ W_out + b_out
    (ones-row matmul bias), out staged via ScalarE copy (Act is idle in the
    drain) and DMA'd per row-chunk, overlapped with the final GEMMs.
"""

import numpy as np

P = 128
N_CTX = 1024
DIM = 512
H = 8
HD = 128
QKV = 3072
SCALE = 0.125  # (512 // 8) ** -0.5, faithful to the reference

_cached_nc = None


def _build_nc(loop_n=1):
    from contextlib import ExitStack

    import concourse.mybir as mybir
    import concourse.tile as tile
    from concourse import bacc
    from concourse.masks import make_identity

    F32 = mybir.dt.float32
    BF16 = mybir.dt.bfloat16
    AF = mybir.ActivationFunctionType

    nc = bacc.Bacc()

    x_ext = nc.declare_dram_parameter("x", [N_CTX, DIM], F32, isOutput=False)
    wqkv_ext = nc.declare_dram_parameter("W_qkv", [DIM, QKV], F32, isOutput=False)
    bqkv_ext = nc.declare_dram_parameter("b_qkv", [QKV], F32, isOutput=False)
    wout_ext = nc.declare_dram_parameter("W_out", [N_CTX, DIM], F32, isOutput=False)
    bout_ext = nc.declare_dram_parameter("b_out", [DIM], F32, isOutput=False)
    out_ext = nc.declare_dram_parameter("out", [N_CTX, DIM], F32, isOutput=True)

    NT = N_CTX // P  # 8 row tiles
    KD = DIM // P  # 4 contraction chunks for dim=512
    VW = HD + 1  # 129: v columns per head incl. ones column

    with ExitStack() as ctx:
        tc = ctx.enter_context(tile.TileContext(nc))
        consts = ctx.enter_context(tc.tile_pool(name="consts", bufs=1))
        persist = ctx.enter_context(tc.tile_pool(name="persist", bufs=1))
        work = ctx.enter_context(tc.tile_pool(name="work", bufs=2))
        small = ctx.enter_context(tc.tile_pool(name="small", bufs=3))
        ps_big = ctx.enter_context(tc.tile_pool(name="ps_big", bufs=2, space="PSUM"))
        ps_bank = ctx.enter_context(tc.tile_pool(name="ps_bank", bufs=4, space="PSUM"))

        # ---- constants / weights (outside any bench loop) -------------------
        ident = consts.tile([P, P], BF16, tag="ident")
        make_identity(nc, ident)
        ones_row = consts.tile([1, P], BF16, tag="ones_row")
        nc.vector.memset(ones_row, 1.0)

        # q/k bias first on the sync queue (tiny), then x
        bqk_sb = consts.tile([P, 2 * H], F32, tag="bqk")
        nc.sync.dma_start(
            bqk_sb, bqkv_ext[0 : 2 * H * P].rearrange("(t p) -> p t", p=P)
        )
        x_sb = persist.tile([P, NT, DIM], F32, tag="x_sb")
        for t in range(NT):
            nc.sync.dma_start(
                x_sb[:, t, :], x_ext.rearrange("(t p) d -> p t d", p=P)[:, t, :]
            )

        # W_qkv as (p, ko, 3072) bf16 — contraction dim on partitions.
        # q|k columns first (head-0 projection starts the pipeline), v after.
        wq_sb = consts.tile([P, KD, QKV], BF16, tag="wq")
        wq_r = wqkv_ext.rearrange("(ko p) n -> p ko n", p=P)
        for part in range(2):  # 0: q cols, 1: k cols
            for k in range(KD):
                sl = slice(part * H * P, (part + 1) * H * P)
                nc.gpsimd.dma_start(wq_sb[:, k, sl], wq_r[:, k, sl])
        for k in range(KD):
            nc.gpsimd.dma_start(wq_sb[:, k, 2 * H * P :], wq_r[:, k, 2 * H * P :])
        # W_out as (p, kh, 512) bf16 — contraction dim (h*hd) on partitions
        wout_sb = consts.tile([P, H, DIM], BF16, tag="wout")
        nc.gpsimd.dma_start(wout_sb, wout_ext.rearrange("(kh p) c -> p kh c", p=P))
        # v bias and out bias as single-partition rows (bf16, for K=1 matmuls)
        bv_row = consts.tile([1, H * HD], BF16, tag="bv")
        nc.gpsimd.dma_start(bv_row, bqkv_ext[2 * H * P : QKV][None, :])
        bout_row = consts.tile([1, DIM], BF16, tag="bout")
        nc.gpsimd.dma_start(bout_row, bout_ext[None, :])

        def body(_iv=None):
            # ---- x^T: cast to bf16 on DVE; PE transposes batched 4 blocks
            # per [128,512] PSUM bank, one wide DVE copy per bank ------------
            x_bf = persist.tile([P, NT, DIM], BF16, tag="x_bf")
            for t in range(NT):
                nc.vector.tensor_copy(x_bf[:, t, :], x_sb[:, t, :])
            xT = persist.tile([P, KD, N_CTX], BF16, tag="xT")
            for c in range(KD):
                for g in range(2):
                    tp = ps_bank.tile([P, 4 * P], BF16, tag="bank")
                    for b in range(4):
                        t = 4 * g + b
                        nc.tensor.transpose(
                            tp[:, b * P : (b + 1) * P],
                            x_bf[:, t, c * P : (c + 1) * P],
                            ident,
                        )
                    nc.vector.tensor_copy(
                        xT[:, c, 4 * g * P : 4 * (g + 1) * P], tp
                    )

            outT = persist.tile([P, H, N_CTX], BF16, tag="outT")
            out_sb = persist.tile([P, NT, DIM], F32, tag="out_sb")
            out_r = out_ext.rearrange("(t p) c -> p t c", p=P)

            def emit_qk_group(h, grp):
                # one of 4 projection groups (part, half) for head h
                part, half = divmod(grp, 2)
                m = part * H + h
                sl = slice(half * DIM, (half + 1) * DIM)
                ps = ps_bank.tile([P, DIM], F32, tag="bank")
                for k in range(KD):
                    nc.tensor.matmul(
                        ps,
                        wq_sb[:, k, m * P : (m + 1) * P],
                        xT[:, k, sl],
                        start=(k == 0),
                        stop=(k == KD - 1),
                    )
                nc.vector.tensor_scalar_add(
                    qk_tiles[(h % 2, part)][:, sl], ps, bqk_sb[:, m : m + 1]
                )

            def emit_scores_j(h, pT, j):
                qT_h = qk_tiles[(h % 2, 0)]
                kT_h = qk_tiles[(h % 2, 1)]
                ps = ps_big.tile([P, N_CTX], F32, tag="big")
                for half in range(2):
                    sl = slice(half * DIM, (half + 1) * DIM)
                    nc.tensor.matmul(
                        ps[:, sl],
                        kT_h[:, j * P : (j + 1) * P],
                        qT_h[:, sl],
                        start=True,
                        stop=True,
                    )
                nc.scalar.activation(pT[:, j, :], ps, AF.Exp, scale=SCALE)

            def emit_attn_ic(h, pT, at_st, ic):
                aps = ps_bank.tile([P, VW], F32, tag="bank")
                for j in range(NT):
                    nc.tensor.matmul(
                        aps[:, :VW],
                        pT[:, j, ic * P : (ic + 1) * P],
                        v_sb[:, j, h * VW : (h + 1) * VW],
                        start=(j == 0),
                        stop=(j == NT - 1),
                    )
                rc = small.tile([P, 1], F32, tag="rc")
                nc.vector.reciprocal(rc, aps[:, HD : HD + 1])
                nc.vector.tensor_scalar_mul(at_st[:, ic, :], aps[:, 0:HD], rc)

            def emit_transp(h, at_st, ic):
                tp = ps_bank.tile([P, P], BF16, tag="bank")
                nc.tensor.transpose(tp, at_st[:, ic, :], ident)
                nc.vector.tensor_copy(outT[:, h, ic * P : (ic + 1) * P], tp)

            def emit_final_ic(ic):
                fps = ps_bank.tile([P, DIM], F32, tag="bank")
                for kh in range(H):
                    nc.tensor.matmul(
                        fps,
                        outT[:, kh, ic * P : (ic + 1) * P],
                        wout_sb[:, kh, :],
                        start=(kh == 0),
                        stop=False,
                    )
                nc.tensor.matmul(fps, ones_row, bout_row, start=False, stop=True)
                nc.scalar.copy(out_sb[:, ic, :], fps)
                nc.sync.dma_start(out_r[:, ic, :], out_sb[:, ic, :])

            # double-buffered q^T/k^T tiles, indexed by (h%2, part)
            qk_tiles = {}
            for hb in range(2):
                for part in range(2):
                    qk_tiles[(hb, part)] = work.tile(
                        [P, N_CTX],
                        BF16,
                        tag=f"qkT{hb}{part}",
                        name=f"qkT{hb}{part}",
                    )

            # ---- fill: head-0 projection + scores, then the V GEMM ---------
            for grp in range(4):
                emit_qk_group(0, grp)
            pT_prev = work.tile([P, NT, N_CTX], BF16, tag="pT")
            for j in range(NT):
                emit_scores_j(0, pT_prev, j)

            for grp in range(4):
                emit_qk_group(1, grp)

            v_sb = persist.tile([P, NT, H * VW], BF16, tag="v_sb")
            nc.vector.memset(
                v_sb.rearrange("p t (h w) -> p t h w", w=VW)[:, :, :, HD : HD + 1],
                1.0,
            )
            for t in range(NT):
                for half in range(2):
                    ps = ps_bank.tile([P, DIM], F32, tag="bank")
                    for k in range(KD):
                        nc.tensor.matmul(
                            ps,
                            xT[:, k, t * P : (t + 1) * P],
                            wq_sb[
                                :,
                                k,
                                2 * H * P + half * DIM : 2 * H * P + (half + 1) * DIM,
                            ],
                            start=(k == 0),
                            stop=False,
                        )
                    nc.tensor.matmul(
                        ps,
                        ones_row,
                        bv_row[:, half * DIM : (half + 1) * DIM],
                        start=False,
                        stop=True,
                    )
                    dst = v_sb[:, t, :].rearrange("p (h w) -> p h w", w=VW)[
                        :, half * 4 : (half + 1) * 4, 0:HD
                    ]
                    src = ps.rearrange("p (h w) -> p h w", w=HD)
                    nc.vector.tensor_copy(dst, src)

            # ---- steady phases h = 1..H+1 ----------------------------------
            # phase h: scores+exp of head h, AV of head h-1, transposes of
            # head h-2, projection of head h+1.
            at_prev = None  # at_store of head h-2 (to transpose this phase)
            at_cur = None
            for h in range(1, H + 2):
                if h <= H:
                    at_cur = work.tile([P, NT, P], BF16, tag="at_store")
                if h < H:
                    pT_cur = work.tile([P, NT, N_CTX], BF16, tag="pT")
                for j in range(NT):
                    if h < H:
                        emit_scores_j(h, pT_cur, j)
                    if h <= H:
                        emit_attn_ic(h - 1, pT_prev, at_cur, j)
                    if h < H - 1 and j % 2 == 0:
                        emit_qk_group(h + 1, j // 2)
                    if at_prev is not None:
                        emit_transp(h - 2, at_prev, j)
                    if h == H + 1 and j >= 1:
                        emit_final_ic(j - 1)
                if h == H + 1:
                    emit_final_ic(NT - 1)
                at_prev = at_cur
                if h < H:
                    pT_prev = pT_cur

        if loop_n == 1:
            body()
        else:
            with tc.For_i(0, loop_n, 1) as iv:
                body(iv)

    nc.finalize()
    return nc


def _get_nc():
    global _cached_nc
    if _cached_nc is None:
        _cached_nc = _build_nc()
    return _cached_nc


def kernel(**inputs):
    from concourse.bass_utils import run_bass_kernel_spmd

    x = np.ascontiguousarray(np.asarray(inputs["x"], dtype=np.float32))
    W_qkv = np.ascontiguousarray(np.asarray(inputs["W_qkv"], dtype=np.float32))
    b_qkv = np.ascontiguousarray(np.asarray(inputs["b_qkv"], dtype=np.float32))
    W_out = np.ascontiguousarray(np.asarray(inputs["W_out"], dtype=np.float32))
    b_out = np.ascontiguousarray(np.asarray(inputs["b_out"], dtype=np.float32))

    bt, b_sz, n, dim = x.shape
    xs = x.reshape(bt * b_sz, n, dim)
    nc = _get_nc()
    in_maps = [
        {
            "x": np.ascontiguousarray(xs[c]),
            "W_qkv": W_qkv,
            "b_qkv": b_qkv,
            "W_out": W_out,
            "b_out": b_out,
        }
        for c in range(8)
    ]
    res = run_bass_kernel_spmd(nc, in_maps, core_ids=list(range(8)))
    outs = np.stack([np.asarray(res.results[c]["out"]) for c in range(8)])
    return outs.reshape(bt, b_sz, n, dim).astype(np.float32)


# revision 15
# speedup vs baseline: 4.0088x; 1.3421x over previous
"""Multi-head attention block kernel for Trainium2 (8 NeuronCores).

Problem: x:(2,4,1024,512) fp32, W_qkv:(512,3072), b_qkv:(3072,),
W_out:(1024,512), b_out:(512,).  out = Attention(x) per (bt,b) item.

Sharding: pure data parallel — bt*b_sz = 8 batch items, one per core.
Each core runs the full attention block on its (1024, 512) slice:
  qkv = x @ W_qkv + b_qkv           (heads=8, hd=128; scale=1/sqrt(64))
  P   = softmax(q*scale @ k^T)
  o   = (P @ v) reshaped, then o @ W_out + b_out

On-chip plan (all matmuls bf16 with fp32 PSUM accumulation).  HW-measured
rates on this part: N=512 MM ~199ns, N=129 MM ~82ns, exp[128,1024] ~1.2us,
and a PE-transpose->DVE-copy round trip costs >1us when the input is hot
(cross-engine ping-pong), but streams cleanly when the input is a phase old.
Schedule (emission order IS the schedule; engines run in-order):
  - fill: x cast (DVE) -> x^T PE-transposed 4 blocks per PSUM bank, one wide
    DVE copy per bank; q^T,k^T of head 0; scores+exp of head 0; THEN the
    V GEMM (its 16us of PE work hides head-0's 8 exps).  W_qkv DMA loads
    q|k columns first so head 0 can start.  V layout: heads side by side
    with a ones column appended (v | 1) -> the attention matmul's column
    128 accumulates the softmax denominator for free.
  - steady phase h: per query-chunk j: scores S^T_j(h) (2 N=512 MMs) ->
    exp on ScalarE; AV chain of head h-1 (8 N=129 MMs, rhs [v|1]) -> DVE
    reciprocal + normalize into at_store[h-1]; PE-transpose of head h-2's
    at_store (input a full phase old -> no ping-pong) -> DVE copy to outT;
    head h+1's q^T,k^T projection MMs spread across the phase (4-MM groups
    after each even j) so its DVE bias-adds land a phase early.
  - drain: transposes of head 7, then final = outT^T @ W_out + b_out
    (ones-row matmul bias), out staged via ScalarE copy (Act is idle in the
    drain) and DMA'd per row-chunk, overlapped with the final GEMMs.
"""

import numpy as np

P = 128
N_CTX = 1024
DIM = 512
H = 8
HD = 128
QKV = 3072
SCALE = 0.125  # (512 // 8) ** -0.5, faithful to the reference

_cached_nc = None


def _build_nc(loop_n=1):
    from contextlib import ExitStack

    import concourse.mybir as mybir
    import concourse.tile as tile
    from concourse import bacc
    from concourse.masks import make_identity

    F32 = mybir.dt.float32
    BF16 = mybir.dt.bfloat16
    AF = mybir.ActivationFunctionType

    nc = bacc.Bacc()

    x_ext = nc.declare_dram_parameter("x", [N_CTX, DIM], F32, isOutput=False)
    wqkv_ext = nc.declare_dram_parameter("W_qkv", [DIM, QKV], F32, isOutput=False)
    bqkv_ext = nc.declare_dram_parameter("b_qkv", [QKV], F32, isOutput=False)
    wout_ext = nc.declare_dram_parameter("W_out", [N_CTX, DIM], F32, isOutput=False)
    bout_ext = nc.declare_dram_parameter("b_out", [DIM], F32, isOutput=False)
    out_ext = nc.declare_dram_parameter("out", [N_CTX, DIM], F32, isOutput=True)

    NT = N_CTX // P  # 8 row tiles
    KD = DIM // P  # 4 contraction chunks for dim=512
    VW = HD + 1  # 129: v columns per head incl. ones column

    with ExitStack() as ctx:
        tc = ctx.enter_context(tile.TileContext(nc))
        consts = ctx.enter_context(tc.tile_pool(name="consts", bufs=1))
        persist = ctx.enter_context(tc.tile_pool(name="persist", bufs=1))
        work = ctx.enter_context(tc.tile_pool(name="work", bufs=2))
        small = ctx.enter_context(tc.tile_pool(name="small", bufs=3))
        ps_big = ctx.enter_context(tc.tile_pool(name="ps_big", bufs=2, space="PSUM"))
        ps_bank = ctx.enter_context(tc.tile_pool(name="ps_bank", bufs=4, space="PSUM"))

        # ---- constants / weights (outside any bench loop) -------------------
        ident = consts.tile([P, P], BF16, tag="ident")
        make_identity(nc, ident)
        ones_row = consts.tile([1, P], BF16, tag="ones_row")
        nc.vector.memset(ones_row, 1.0)

        # q/k bias first on the sync queue (tiny), then x
        bqk_sb = consts.tile([P, 2 * H], F32, tag="bqk")
        nc.sync.dma_start(
            bqk_sb, bqkv_ext[0 : 2 * H * P].rearrange("(t p) -> p t", p=P)
        )
        x_sb = persist.tile([P, NT, DIM], F32, tag="x_sb")
        for t in range(NT):
            nc.sync.dma_start(
                x_sb[:, t, :], x_ext.rearrange("(t p) d -> p t d", p=P)[:, t, :]
            )

        # v bias and out bias rows first on the gpsimd queue (tiny) — the
        # PE's in-order stream starts with the broadcast matmuls below and
        # must not wait behind the big weight loads.
        bv_row = consts.tile([1, H * HD], BF16, tag="bv")
        nc.gpsimd.dma_start(bv_row, bqkv_ext[2 * H * P : QKV][None, :])
        bout_row = consts.tile([1, DIM], F32, tag="bout")
        nc.gpsimd.dma_start(bout_row, bout_ext[None, :])
        # W_qkv as (p, ko, 3072) bf16 — contraction dim on partitions.
        # q|k columns first (head-0 projection starts the pipeline), v after.
        wq_sb = consts.tile([P, KD, QKV], BF16, tag="wq")
        wq_r = wqkv_ext.rearrange("(ko p) n -> p ko n", p=P)
        for part in range(2):  # 0: q cols, 1: k cols
            for k in range(KD):
                sl = slice(part * H * P, (part + 1) * H * P)
                nc.gpsimd.dma_start(wq_sb[:, k, sl], wq_r[:, k, sl])
        for k in range(KD):
            nc.gpsimd.dma_start(wq_sb[:, k, 2 * H * P :], wq_r[:, k, 2 * H * P :])
        # W_out as (p, kh, 512) bf16 — contraction dim (h*hd) on partitions
        wout_sb = consts.tile([P, H, DIM], BF16, tag="wout")
        nc.gpsimd.dma_start(wout_sb, wout_ext.rearrange("(kh p) c -> p kh c", p=P))
        # bias broadcasts to all 128 partitions (outside the loop): ones-
        # column matmul, then copy PSUM -> SBUF.  The per-tile bias adds then
        # fuse into the PSUM->SBUF copies as tensor_tensor adds instead of
        # costing K=1 matmuls per accumulation group.
        ones_col = consts.tile([1, P], BF16, tag="ones_col")
        nc.vector.memset(ones_col, 1.0)
        bv_bc = consts.tile([P, H * HD], BF16, tag="bv_bc")
        for half in range(2):
            sl = slice(half * DIM, (half + 1) * DIM)
            bps = ps_bank.tile([P, DIM], F32, tag="bank")
            nc.tensor.matmul(
                bps, ones_col, bv_row[:, sl], start=True, stop=True
            )
            nc.vector.tensor_copy(bv_bc[:, sl], bps)
        bout_bc = consts.tile([P, DIM], F32, tag="bout_bc")
        bout_bf = consts.tile([1, DIM], BF16, tag="bout_bf")
        nc.vector.tensor_copy(bout_bf, bout_row)
        bps = ps_bank.tile([P, DIM], F32, tag="bank")
        nc.tensor.matmul(bps, ones_col, bout_bf, start=True, stop=True)
        nc.vector.tensor_copy(bout_bc, bps)

        def body(_iv=None):
            # ---- x^T: cast to bf16 on DVE; PE transposes batched 4 blocks
            # per [128,512] PSUM bank, one wide DVE copy per bank ------------
            x_bf = persist.tile([P, NT, DIM], BF16, tag="x_bf")
            for t in range(NT):
                nc.vector.tensor_copy(x_bf[:, t, :], x_sb[:, t, :])
            xT = persist.tile([P, KD, N_CTX], BF16, tag="xT")
            for c in range(KD):
                for g in range(2):
                    tp = ps_bank.tile([P, 4 * P], BF16, tag="bank")
                    for b in range(4):
                        t = 4 * g + b
                        nc.tensor.transpose(
                            tp[:, b * P : (b + 1) * P],
                            x_bf[:, t, c * P : (c + 1) * P],
                            ident,
                        )
                    nc.vector.tensor_copy(
                        xT[:, c, 4 * g * P : 4 * (g + 1) * P], tp
                    )

            outT = persist.tile([P, H, N_CTX], BF16, tag="outT")
            out_sb = persist.tile([P, NT, DIM], F32, tag="out_sb")
            out_r = out_ext.rearrange("(t p) c -> p t c", p=P)

            def emit_qk_group(h, grp):
                # one of 4 projection groups (part, half) for head h
                part, half = divmod(grp, 2)
                m = part * H + h
                sl = slice(half * DIM, (half + 1) * DIM)
                ps = ps_bank.tile([P, DIM], F32, tag="bank")
                for k in range(KD):
                    nc.tensor.matmul(
                        ps,
                        wq_sb[:, k, m * P : (m + 1) * P],
                        xT[:, k, sl],
                        start=(k == 0),
                        stop=(k == KD - 1),
                    )
                nc.vector.tensor_scalar_add(
                    qk_tiles[(h % 2, part)][:, sl], ps, bqk_sb[:, m : m + 1]
                )

            def emit_scores_j(h, pT, j):
                qT_h = qk_tiles[(h % 2, 0)]
                kT_h = qk_tiles[(h % 2, 1)]
                ps = ps_big.tile([P, N_CTX], F32, tag="big")
                for half in range(2):
                    sl = slice(half * DIM, (half + 1) * DIM)
                    nc.tensor.matmul(
                        ps[:, sl],
                        kT_h[:, j * P : (j + 1) * P],
                        qT_h[:, sl],
                        start=True,
                        stop=True,
                    )
                nc.scalar.activation(pT[:, j, :], ps, AF.Exp, scale=SCALE)

            def emit_attn_ic(h, pT, at_st, ic):
                aps = ps_bank.tile([P, VW], F32, tag="bank")
                for j in range(NT):
                    nc.tensor.matmul(
                        aps[:, :VW],
                        pT[:, j, ic * P : (ic + 1) * P],
                        v_sb[:, j, h * VW : (h + 1) * VW],
                        start=(j == 0),
                        stop=(j == NT - 1),
                    )
                rc = small.tile([P, 1], F32, tag="rc")
                nc.vector.reciprocal(rc, aps[:, HD : HD + 1])
                nc.vector.tensor_scalar_mul(at_st[:, ic, :], aps[:, 0:HD], rc)

            def emit_transp(h, at_st, ic):
                tp = ps_bank.tile([P, P], BF16, tag="bank")
                nc.tensor.transpose(tp, at_st[:, ic, :], ident)
                nc.vector.tensor_copy(outT[:, h, ic * P : (ic + 1) * P], tp)

            def emit_final_ic(ic):
                fps = ps_bank.tile([P, DIM], F32, tag="bank")
                for kh in range(H):
                    nc.tensor.matmul(
                        fps,
                        outT[:, kh, ic * P : (ic + 1) * P],
                        wout_sb[:, kh, :],
                        start=(kh == 0),
                        stop=(kh == H - 1),
                    )
                nc.vector.tensor_add(out_sb[:, ic, :], fps, bout_bc)
                nc.sync.dma_start(out_r[:, ic, :], out_sb[:, ic, :])

            # double-buffered q^T/k^T tiles, indexed by (h%2, part)
            qk_tiles = {}
            for hb in range(2):
                for part in range(2):
                    qk_tiles[(hb, part)] = work.tile(
                        [P, N_CTX],
                        BF16,
                        tag=f"qkT{hb}{part}",
                        name=f"qkT{hb}{part}",
                    )

            # ---- fill: head-0 projection + scores, then the V GEMM ---------
            for grp in range(4):
                emit_qk_group(0, grp)
            pT_prev = work.tile([P, NT, N_CTX], BF16, tag="pT")
            for j in range(NT):
                emit_scores_j(0, pT_prev, j)

            for grp in range(4):
                emit_qk_group(1, grp)

            v_sb = persist.tile([P, NT, H * VW], BF16, tag="v_sb")
            nc.vector.memset(
                v_sb.rearrange("p t (h w) -> p t h w", w=VW)[:, :, :, HD : HD + 1],
                1.0,
            )
            for t in range(NT):
                for half in range(2):
                    ps = ps_bank.tile([P, DIM], F32, tag="bank")
                    for k in range(KD):
                        nc.tensor.matmul(
                            ps,
                            xT[:, k, t * P : (t + 1) * P],
                            wq_sb[
                                :,
                                k,
                                2 * H * P + half * DIM : 2 * H * P + (half + 1) * DIM,
                            ],
                            start=(k == 0),
                            stop=(k == KD - 1),
                        )
                    dst = v_sb[:, t, :].rearrange("p (h w) -> p h w", w=VW)[
                        :, half * 4 : (half + 1) * 4, 0:HD
                    ]
                    src = ps.rearrange("p (h w) -> p h w", w=HD)
                    bvs = bv_bc[:, half * DIM : (half + 1) * DIM].rearrange(
                        "p (h w) -> p h w", w=HD
                    )
                    nc.vector.tensor_add(dst, src, bvs)

            # ---- steady phases h = 1..H+1 ----------------------------------
            # phase h: scores+exp of head h, AV of head h-1, transposes of
            # head h-2, projection of head h+1.
            at_prev = None  # at_store of head h-2 (to transpose this phase)
            at_cur = None
            for h in range(1, H + 2):
                if h <= H:
                    at_cur = work.tile([P, NT, P], BF16, tag="at_store")
                if h < H:
                    pT_cur = work.tile([P, NT, N_CTX], BF16, tag="pT")
                for j in range(NT):
                    if h < H:
                        emit_scores_j(h, pT_cur, j)
                    if h <= H:
                        emit_attn_ic(h - 1, pT_prev, at_cur, j)
                    if h < H - 1 and j % 2 == 0:
                        emit_qk_group(h + 1, j // 2)
                    if at_prev is not None:
                        emit_transp(h - 2, at_prev, j)
                    if h == H + 1 and j >= 1:
                        emit_final_ic(j - 1)
                if h == H + 1:
                    emit_final_ic(NT - 1)
                at_prev = at_cur
                if h < H:
                    pT_prev = pT_cur

        if loop_n == 1:
            body()
        else:
            with tc.For_i(0, loop_n, 1) as iv:
                body(iv)

    nc.finalize()
    return nc


def _get_nc():
    global _cached_nc
    if _cached_nc is None:
        _cached_nc = _build_nc()
    return _cached_nc


def kernel(**inputs):
    from concourse.bass_utils import run_bass_kernel_spmd

    x = np.ascontiguousarray(np.asarray(inputs["x"], dtype=np.float32))
    W_qkv = np.ascontiguousarray(np.asarray(inputs["W_qkv"], dtype=np.float32))
    b_qkv = np.ascontiguousarray(np.asarray(inputs["b_qkv"], dtype=np.float32))
    W_out = np.ascontiguousarray(np.asarray(inputs["W_out"], dtype=np.float32))
    b_out = np.ascontiguousarray(np.asarray(inputs["b_out"], dtype=np.float32))

    bt, b_sz, n, dim = x.shape
    xs = x.reshape(bt * b_sz, n, dim)
    nc = _get_nc()
    in_maps = [
        {
            "x": np.ascontiguousarray(xs[c]),
            "W_qkv": W_qkv,
            "b_qkv": b_qkv,
            "W_out": W_out,
            "b_out": b_out,
        }
        for c in range(8)
    ]
    res = run_bass_kernel_spmd(nc, in_maps, core_ids=list(range(8)))
    outs = np.stack([np.asarray(res.results[c]["out"]) for c in range(8)])
    return outs.reshape(bt, b_sz, n, dim).astype(np.float32)


# revision 17
# speedup vs baseline: 4.1106x; 1.0254x over previous
"""Multi-head attention block kernel for Trainium2 (8 NeuronCores).

Problem: x:(2,4,1024,512) fp32, W_qkv:(512,3072), b_qkv:(3072,),
W_out:(1024,512), b_out:(512,).  out = Attention(x) per (bt,b) item.

Sharding: pure data parallel — bt*b_sz = 8 batch items, one per core.
Each core runs the full attention block on its (1024, 512) slice:
  qkv = x @ W_qkv + b_qkv           (heads=8, hd=128; scale=1/sqrt(64))
  P   = softmax(q*scale @ k^T)
  o   = (P @ v) reshaped, then o @ W_out + b_out

On-chip plan (all matmuls bf16 with fp32 PSUM accumulation).  HW-measured
rates on this part: N=512 MM ~199ns, N=129 MM ~82ns, exp[128,1024] ~1.2us,
and a PE-transpose->DVE-copy round trip costs >1us when the input is hot
(cross-engine ping-pong), but streams cleanly when the input is a phase old.
Schedule (emission order IS the schedule; engines run in-order):
  - fill: x cast (DVE) -> x^T PE-transposed 4 blocks per PSUM bank, one wide
    DVE copy per bank; q^T,k^T of head 0; scores+exp of head 0; THEN the
    V GEMM (its 16us of PE work hides head-0's 8 exps).  W_qkv DMA loads
    q|k columns first so head 0 can start.  V layout: heads side by side
    with a ones column appended (v | 1) -> the attention matmul's column
    128 accumulates the softmax denominator for free.
  - steady phase h: per query-chunk j: scores S^T_j(h) (2 N=512 MMs) ->
    exp on ScalarE; AV chain of head h-1 (8 N=129 MMs, rhs [v|1]) -> DVE
    reciprocal + normalize into at_store[h-1]; PE-transpose of head h-2's
    at_store (input a full phase old -> no ping-pong) -> DVE copy to outT;
    head h+1's q^T,k^T projection MMs spread across the phase (4-MM groups
    after each even j) so its DVE bias-adds land a phase early.
  - drain: transposes of head 7, then final = outT^T @ W_out + b_out
    (ones-row matmul bias), out staged via ScalarE copy (Act is idle in the
    drain) and DMA'd per row-chunk, overlapped with the final GEMMs.
"""

import numpy as np

P = 128
N_CTX = 1024
DIM = 512
H = 8
HD = 128
QKV = 3072
SCALE = 0.125  # (512 // 8) ** -0.5, faithful to the reference

_cached_nc = None


def _build_nc(loop_n=1):
    from contextlib import ExitStack

    import concourse.mybir as mybir
    import concourse.tile as tile
    from concourse import bacc
    from concourse.masks import make_identity

    F32 = mybir.dt.float32
    BF16 = mybir.dt.bfloat16
    AF = mybir.ActivationFunctionType

    nc = bacc.Bacc()

    x_ext = nc.declare_dram_parameter("x", [N_CTX, DIM], F32, isOutput=False)
    wqkv_ext = nc.declare_dram_parameter("W_qkv", [DIM, QKV], F32, isOutput=False)
    bqkv_ext = nc.declare_dram_parameter("b_qkv", [QKV], F32, isOutput=False)
    wout_ext = nc.declare_dram_parameter("W_out", [N_CTX, DIM], F32, isOutput=False)
    bout_ext = nc.declare_dram_parameter("b_out", [DIM], F32, isOutput=False)
    out_ext = nc.declare_dram_parameter("out", [N_CTX, DIM], F32, isOutput=True)

    NT = N_CTX // P  # 8 row tiles
    KD = DIM // P  # 4 contraction chunks for dim=512
    VW = HD + 1  # 129: v columns per head incl. ones column

    with ExitStack() as ctx:
        tc = ctx.enter_context(tile.TileContext(nc))
        consts = ctx.enter_context(tc.tile_pool(name="consts", bufs=1))
        persist = ctx.enter_context(tc.tile_pool(name="persist", bufs=1))
        work = ctx.enter_context(tc.tile_pool(name="work", bufs=2))
        small = ctx.enter_context(tc.tile_pool(name="small", bufs=3))
        ps_big = ctx.enter_context(tc.tile_pool(name="ps_big", bufs=2, space="PSUM"))
        ps_bank = ctx.enter_context(tc.tile_pool(name="ps_bank", bufs=4, space="PSUM"))

        # ---- constants / weights (outside any bench loop) -------------------
        ident = consts.tile([P, P], BF16, tag="ident")
        make_identity(nc, ident)
        ones_row = consts.tile([1, P], BF16, tag="ones_row")
        nc.vector.memset(ones_row, 1.0)

        # q/k bias first on the sync queue (tiny), then x
        bqk_sb = consts.tile([P, 2 * H], F32, tag="bqk")
        nc.sync.dma_start(
            bqk_sb, bqkv_ext[0 : 2 * H * P].rearrange("(t p) -> p t", p=P)
        )
        x_sb = persist.tile([P, NT, DIM], F32, tag="x_sb")
        for t in range(NT):
            nc.sync.dma_start(
                x_sb[:, t, :], x_ext.rearrange("(t p) d -> p t d", p=P)[:, t, :]
            )

        # v bias and out bias rows first on the gpsimd queue (tiny) — the
        # PE's in-order stream starts with the broadcast matmuls below and
        # must not wait behind the big weight loads.
        bv_row = consts.tile([1, H * HD], BF16, tag="bv")
        nc.gpsimd.dma_start(bv_row, bqkv_ext[2 * H * P : QKV][None, :])
        bout_row = consts.tile([1, DIM], F32, tag="bout")
        nc.gpsimd.dma_start(bout_row, bout_ext[None, :])
        # W_qkv as (p, ko, 3072) bf16 — contraction dim on partitions.
        # q|k columns first (head-0 projection starts the pipeline), v after.
        wq_sb = consts.tile([P, KD, QKV], BF16, tag="wq")
        wq_r = wqkv_ext.rearrange("(ko p) n -> p ko n", p=P)
        for part in range(2):  # 0: q cols, 1: k cols
            for k in range(KD):
                sl = slice(part * H * P, (part + 1) * H * P)
                nc.gpsimd.dma_start(wq_sb[:, k, sl], wq_r[:, k, sl])
        for k in range(KD):
            nc.gpsimd.dma_start(wq_sb[:, k, 2 * H * P :], wq_r[:, k, 2 * H * P :])
        # W_out as (p, kh, 512) bf16 — contraction dim (h*hd) on partitions
        wout_sb = consts.tile([P, H, DIM], BF16, tag="wout")
        nc.gpsimd.dma_start(wout_sb, wout_ext.rearrange("(kh p) c -> p kh c", p=P))
        # bias broadcasts to all 128 partitions (outside the loop): ones-
        # column matmul, then copy PSUM -> SBUF.  The per-tile bias adds then
        # fuse into the PSUM->SBUF copies as tensor_tensor adds instead of
        # costing K=1 matmuls per accumulation group.
        ones_col = consts.tile([1, P], BF16, tag="ones_col")
        nc.vector.memset(ones_col, 1.0)
        bv_bc = consts.tile([P, H * HD], BF16, tag="bv_bc")
        for half in range(2):
            sl = slice(half * DIM, (half + 1) * DIM)
            bps = ps_bank.tile([P, DIM], F32, tag="bank")
            nc.tensor.matmul(
                bps, ones_col, bv_row[:, sl], start=True, stop=True
            )
            nc.vector.tensor_copy(bv_bc[:, sl], bps)
        bout_bc = consts.tile([P, DIM], F32, tag="bout_bc")
        bout_bf = consts.tile([1, DIM], BF16, tag="bout_bf")
        nc.vector.tensor_copy(bout_bf, bout_row)
        bps = ps_bank.tile([P, DIM], F32, tag="bank")
        nc.tensor.matmul(bps, ones_col, bout_bf, start=True, stop=True)
        nc.vector.tensor_copy(bout_bc, bps)

        def body(_iv=None):
            # ---- x^T: cast to bf16 on DVE; PE transposes batched 4 blocks
            # per [128,512] PSUM bank, one wide DVE copy per bank ------------
            x_bf = persist.tile([P, NT, DIM], BF16, tag="x_bf")
            for t in range(NT):
                # split casts across DVE and Act so the first transpose
                # group's four input tiles are ready in half the time
                if t % 2 == 0:
                    nc.vector.tensor_copy(x_bf[:, t, :], x_sb[:, t, :])
                else:
                    nc.scalar.copy(x_bf[:, t, :], x_sb[:, t, :])
            xT = persist.tile([P, KD, N_CTX], BF16, tag="xT")
            for c in range(KD):
                for g in range(2):
                    tp = ps_bank.tile([P, 4 * P], BF16, tag="bank")
                    for b in range(4):
                        t = 4 * g + b
                        nc.tensor.transpose(
                            tp[:, b * P : (b + 1) * P],
                            x_bf[:, t, c * P : (c + 1) * P],
                            ident,
                        )
                    nc.vector.tensor_copy(
                        xT[:, c, 4 * g * P : 4 * (g + 1) * P], tp
                    )

            outT = persist.tile([P, H, N_CTX], BF16, tag="outT")
            out_sb = persist.tile([P, NT, DIM], F32, tag="out_sb")
            out_r = out_ext.rearrange("(t p) c -> p t c", p=P)

            def emit_qk_group(h, grp):
                # one of 4 projection groups (part, half) for head h
                part, half = divmod(grp, 2)
                m = part * H + h
                sl = slice(half * DIM, (half + 1) * DIM)
                ps = ps_bank.tile([P, DIM], F32, tag="bank")
                for k in range(KD):
                    nc.tensor.matmul(
                        ps,
                        wq_sb[:, k, m * P : (m + 1) * P],
                        xT[:, k, sl],
                        start=(k == 0),
                        stop=(k == KD - 1),
                    )
                nc.vector.tensor_scalar_add(
                    qk_tiles[(h % 2, part)][:, sl], ps, bqk_sb[:, m : m + 1]
                )

            def emit_scores_j(h, pT, j):
                qT_h = qk_tiles[(h % 2, 0)]
                kT_h = qk_tiles[(h % 2, 1)]
                ps = ps_big.tile([P, N_CTX], F32, tag="big")
                for half in range(2):
                    sl = slice(half * DIM, (half + 1) * DIM)
                    nc.tensor.matmul(
                        ps[:, sl],
                        kT_h[:, j * P : (j + 1) * P],
                        qT_h[:, sl],
                        start=True,
                        stop=True,
                    )
                nc.scalar.activation(pT[:, j, :], ps, AF.Exp, scale=SCALE)

            def emit_attn_ic(h, pT, at_st, ic):
                aps = ps_bank.tile([P, VW], F32, tag="bank")
                for j in range(NT):
                    nc.tensor.matmul(
                        aps[:, :VW],
                        pT[:, j, ic * P : (ic + 1) * P],
                        v_sb[:, j, h * VW : (h + 1) * VW],
                        start=(j == 0),
                        stop=(j == NT - 1),
                    )
                rc = small.tile([P, 1], F32, tag="rc")
                nc.vector.reciprocal(rc, aps[:, HD : HD + 1])
                nc.vector.tensor_scalar_mul(at_st[:, ic, :], aps[:, 0:HD], rc)

            def emit_transp(h, at_st, ic):
                tp = ps_bank.tile([P, P], BF16, tag="bank")
                nc.tensor.transpose(tp, at_st[:, ic, :], ident)
                nc.vector.tensor_copy(outT[:, h, ic * P : (ic + 1) * P], tp)

            def emit_final_ic(ic):
                fps = ps_bank.tile([P, DIM], F32, tag="bank")
                for kh in range(H):
                    nc.tensor.matmul(
                        fps,
                        outT[:, kh, ic * P : (ic + 1) * P],
                        wout_sb[:, kh, :],
                        start=(kh == 0),
                        stop=(kh == H - 1),
                    )
                nc.vector.tensor_add(out_sb[:, ic, :], fps, bout_bc)
                nc.sync.dma_start(out_r[:, ic, :], out_sb[:, ic, :])

            # double-buffered q^T/k^T tiles, indexed by (h%2, part)
            qk_tiles = {}
            for hb in range(2):
                for part in range(2):
                    qk_tiles[(hb, part)] = work.tile(
                        [P, N_CTX],
                        BF16,
                        tag=f"qkT{hb}{part}",
                        name=f"qkT{hb}{part}",
                    )

            # ---- fill: head-0 projection + scores, then the V GEMM ---------
            for grp in range(4):
                emit_qk_group(0, grp)
            pT_prev = work.tile([P, NT, N_CTX], BF16, tag="pT")
            for j in range(NT):
                emit_scores_j(0, pT_prev, j)

            for grp in range(4):
                emit_qk_group(1, grp)

            v_sb = persist.tile([P, NT, H * VW], BF16, tag="v_sb")
            nc.vector.memset(
                v_sb.rearrange("p t (h w) -> p t h w", w=VW)[:, :, :, HD : HD + 1],
                1.0,
            )
            for t in range(NT):
                for half in range(2):
                    ps = ps_bank.tile([P, DIM], F32, tag="bank")
                    for k in range(KD):
                        nc.tensor.matmul(
                            ps,
                            xT[:, k, t * P : (t + 1) * P],
                            wq_sb[
                                :,
                                k,
                                2 * H * P + half * DIM : 2 * H * P + (half + 1) * DIM,
                            ],
                            start=(k == 0),
                            stop=(k == KD - 1),
                        )
                    dst = v_sb[:, t, :].rearrange("p (h w) -> p h w", w=VW)[
                        :, half * 4 : (half + 1) * 4, 0:HD
                    ]
                    src = ps.rearrange("p (h w) -> p h w", w=HD)
                    bvs = bv_bc[:, half * DIM : (half + 1) * DIM].rearrange(
                        "p (h w) -> p h w", w=HD
                    )
                    nc.vector.tensor_add(dst, src, bvs)

            # ---- steady phases h = 1..H+1 ----------------------------------
            # phase h: scores+exp of head h, AV of head h-1, transposes of
            # head h-2, projection of head h+1.
            at_prev = None  # at_store of head h-2 (to transpose this phase)
            at_cur = None
            for h in range(1, H + 2):
                if h <= H:
                    at_cur = work.tile([P, NT, P], BF16, tag="at_store")
                if h < H:
                    pT_cur = work.tile([P, NT, N_CTX], BF16, tag="pT")
                for j in range(NT):
                    if h < H:
                        emit_scores_j(h, pT_cur, j)
                    if h <= H:
                        emit_attn_ic(h - 1, pT_prev, at_cur, j)
                    if h < H - 1 and j % 2 == 0 and not (h == H - 2 and j == 6):
                        emit_qk_group(h + 1, j // 2)
                    if h == H - 1 and j == 0:
                        # head 7's last projection group deferred into this
                        # otherwise Act-bound phase (used from j=4 on)
                        emit_qk_group(H - 1, 3)
                    if at_prev is not None:
                        emit_transp(h - 2, at_prev, j)
                    if h == H + 1 and j >= 1:
                        emit_final_ic(j - 1)
                if h == H + 1:
                    emit_final_ic(NT - 1)
                at_prev = at_cur
                if h < H:
                    pT_prev = pT_cur

        if loop_n == 1:
            body()
        else:
            with tc.For_i(0, loop_n, 1) as iv:
                body(iv)

    nc.finalize()
    return nc


def _get_nc():
    global _cached_nc
    if _cached_nc is None:
        _cached_nc = _build_nc()
    return _cached_nc


def kernel(**inputs):
    from concourse.bass_utils import run_bass_kernel_spmd

    x = np.ascontiguousarray(np.asarray(inputs["x"], dtype=np.float32))
    W_qkv = np.ascontiguousarray(np.asarray(inputs["W_qkv"], dtype=np.float32))
    b_qkv = np.ascontiguousarray(np.asarray(inputs["b_qkv"], dtype=np.float32))
    W_out = np.ascontiguousarray(np.asarray(inputs["W_out"], dtype=np.float32))
    b_out = np.ascontiguousarray(np.asarray(inputs["b_out"], dtype=np.float32))

    bt, b_sz, n, dim = x.shape
    xs = x.reshape(bt * b_sz, n, dim)
    nc = _get_nc()
    in_maps = [
        {
            "x": np.ascontiguousarray(xs[c]),
            "W_qkv": W_qkv,
            "b_qkv": b_qkv,
            "W_out": W_out,
            "b_out": b_out,
        }
        for c in range(8)
    ]
    res = run_bass_kernel_spmd(nc, in_maps, core_ids=list(range(8)))
    outs = np.stack([np.asarray(res.results[c]["out"]) for c in range(8)])
    return outs.reshape(bt, b_sz, n, dim).astype(np.float32)
